# revision 28
# baseline (speedup 1.0000x reference)
"""Distributed multi-head attention for TRN2 (8 NeuronCores).

Reference computation (per batch b):
    qkv = x @ w_qkv.T                         # (N, 3C)
    q, k, v = split/reshape to (H, N, D)
    attn = softmax(q @ k.T * D**-0.5)         # per head
    out = (attn @ v) reassembled to (N, C)
    out = out @ w_proj.T + b_proj

Sharding: 8 cores = 4 batches x 2 query-halves. Each core computes k/v
for all 2048 tokens of its batch (duplicated across the 2 cores of a
batch - cheaper than communicating), q for its own 1024 tokens, the
full attention for all 12 heads over its 1024 queries, and the output
projection. No collectives.

Layout strategy (all chosen so no on-chip transposes are needed):
  - host passes x^T and w_qkv^T so projections contract over partitions
  - q,k are produced "d-major" ([head-dim, tokens]) via out^T-form
    matmuls; scores are computed transposed ([keys, queries]) which is
    exactly the layout attn@v consumes as its stationary-side operand
  - softmax needs no max-subtraction (scores ~ N(0,1), fp32 exp range)
  - the denominator rides along as a ones-column appended to v (M=65
    matmuls); normalization uses a K=1 ones-matmul to broadcast 1/denom
    across partitions
  - all matmuls in bf16 (PSUM accumulation is fp32); softmax exp runs
    on the scalar (ACT) engine from PSUM f32, writing bf16 probs

Schedule: the ACT engine (softmax exp, ~1us per 128x1024 tile) is the
steady-state bottleneck; everything else hides under it. Attention runs
as 12 passes (head pair x query half). Per pass and k-block: the two
heads' score matmuls write one shared PSUM tile, alternating PE row
groups (base partition 0/64) so they run concurrently; exp(kb) overlaps
scores(kb+1) via two PSUM slots; attn@v lags by one k-block. The query
halving keeps the pass's PSUM footprint at 6 banks, leaving 2 banks for
"filler" projection work that keeps the PE busy (and its HAM clock
warm): pass 0 produces v block kb just-in-time in step kb, passes 1-5
drain the k/q blocks of later pairs. The per-pass normalization
epilogue is split so its PE part lands inside the next pass.

Self-contained: hardcodes B=4, N=2048, C=768, H=12, D=64.
"""

import numpy as np
import ml_dtypes

import concourse.bass as bass
import concourse.mybir as mybir
from concourse import bacc
from concourse.tile import TileContext
from concourse.bass_utils import run_bass_kernel_spmd

F32 = mybir.dt.float32
BF16 = mybir.dt.bfloat16
EXP = mybir.ActivationFunctionType.Exp

B, N, C = 4, 2048, 768
H, D = 12, 64
SCALE = float(D) ** -0.5  # 0.125
NQ = N // 2  # queries per core: 1024
CB = C // 128  # 6 c-chunks
TB = N // 128  # 16 token blocks
HB = H // 2  # 6 head pairs
VW = H * (D + 1)  # 780: v block width with ones columns

N_CORES = 8

# w_qkv columns, grouped in the order the projection units consume them:
# pair-0 k/q, all v, then k/q for pairs 1..5. Each group holds its column
# range for all six 128-row input chunks, contiguously.
_WQ_GROUPS = [(C, 128), (0, 128), (2 * C, C)]
for _ob in range(1, CB):
    _WQ_GROUPS.append((C + _ob * 128, 128))
    _WQ_GROUPS.append((_ob * 128, 128))
_WQ_BASE = {}
_cur = 0
for _o0, _w in _WQ_GROUPS:
    _WQ_BASE[_o0] = (_cur, _w)
    _cur += CB * _w


def _build():
    nc = bacc.Bacc(None, target_bir_lowering=False)

    # host-packed SBUF images: xTp cols = [tch][ci][t]; wqp cols grouped
    # in consumption order (see _WQ_GROUPS)
    xTp = nc.declare_dram_parameter("xTp", [128, CB * N], BF16, isOutput=False)
    wqp = nc.declare_dram_parameter("wqp", [128, CB * 3 * C], BF16, isOutput=False)
    wprojT = nc.declare_dram_parameter("wprojT", [C, C], BF16, isOutput=False)
    bias = nc.declare_dram_parameter("bias", [C, 1], F32, isOutput=False)
    outT = nc.declare_dram_parameter("outT", [C, NQ], BF16, isOutput=True)

    with TileContext(nc) as tc:
        with (
            tc.tile_pool(name="per", bufs=1) as per,
            tc.tile_pool(name="p23", bufs=1) as p23,
            tc.tile_pool(name="hp", bufs=8) as hp,
            tc.tile_pool(name="mi", bufs=3) as mi,
            tc.tile_pool(name="op", bufs=2) as op_pool,
            tc.tile_pool(name="ps", bufs=2, space="PSUM") as ps2,
        ):
            # ---- persistent tiles -------------------------------------
            qT_sb = per.tile([128, CB * NQ], BF16)  # q^T  [2 heads/blk, 1024]
            kT_sb = per.tile([128, CB * N], BF16)  # k^T  [2 heads/blk, 2048]
            vaug_sb = per.tile([128, TB * VW], BF16)  # v + ones cols
            bias_sb = per.tile([128, CB], F32)
            ones_sb = per.tile([1, 64], BF16)
            attnT_sb = p23.tile([128, CB * NQ], BF16)  # attn out^T
            wproj_sb = p23.tile([128, CB * C], BF16)

            nc.vector.memset(ones_sb[:, :], 1.0)
            # ones columns of vaug: col 64 of each 65-wide head slot
            vaug_ones = vaug_sb[:, :].rearrange(
                "p (t h x) -> p t h x", t=TB, h=H, x=D + 1
            )[:, :, :, D : D + 1]
            nc.vector.memset(vaug_ones, 1.0)

            # weights + activations pools, closed once the projection
            # filler has consumed them
            wqxt = (tc.tile_pool(name="wq", bufs=1), tc.tile_pool(name="xt", bufs=4))
            wq_pool = wqxt[0].__enter__()
            xt_pool = wqxt[1].__enter__()

            wqkv_sb = wq_pool.tile([128, CB * 3 * C], BF16)
            xts = [
                xt_pool.tile([128, CB * 512], BF16, tag="xt", name=f"xt{t}")
                for t in range(4)
            ]

            def _dma_xt(tch, parts=3):
                w = CB * 512 // parts
                for i in range(parts):
                    a = tch * CB * 512 + i * w
                    nc.sync.dma_start(
                        out=xts[tch][:, i * w : (i + 1) * w],
                        in_=xTp[:, a : a + w],
                    )

            def _dma_wq(gi, parts=1):
                o0, w = _WQ_GROUPS[gi]
                base, _ = _WQ_BASE[o0]
                gw = CB * w // parts
                for i in range(parts):
                    nc.sync.dma_start(
                        out=wqkv_sb[:, base + i * gw : base + (i + 1) * gw],
                        in_=wqp[:, base + i * gw : base + (i + 1) * gw],
                    )

            # consumption order: chunk 0 + pair-0 k/q cols first, then the
            # remaining token chunks, v cols, later pairs' k/q cols
            _dma_xt(0, parts=6)
            _dma_wq(0, parts=2)
            _dma_wq(1, parts=2)
            for t in range(1, 4):
                _dma_xt(t, parts=3)
            _dma_wq(2, parts=4)  # v columns (1.1 MB)
            for gi in range(3, len(_WQ_GROUPS)):
                _dma_wq(gi)

            def wq(ci, o0, width):
                if o0 >= 2 * C:
                    base, gw = _WQ_BASE[2 * C]
                    off = o0 - 2 * C
                else:
                    base, gw = _WQ_BASE[o0]
                    off = 0
                return wqkv_sb[:, base + ci * gw + off : base + ci * gw + off + width]

            # phase-2/3-only weights: after the critical-path DMAs
            for ci in range(CB):
                nc.sync.dma_start(
                    out=bias_sb[:, ci : ci + 1],
                    in_=bias[ci * 128 : (ci + 1) * 128, :],
                )
                nc.sync.dma_start(
                    out=wproj_sb[:, ci * C : (ci + 1) * C],
                    in_=wprojT[ci * 128 : (ci + 1) * 128, :],
                )

            # ---- projection work units (PE filler) --------------------
            def kq_unit(ob, tch, is_q):
                """one k^T (or q^T) block: out-dims block ob, 512 tokens"""
                t0 = tch * 512
                kind = "q" if is_q else "k"
                psv = ps2.tile(
                    [128, 512], F32, tag="psV", bufs=2, name=f"{kind}{ob}_{tch}"
                )
                for ci in range(CB):
                    nc.tensor.matmul(
                        psv[:, :],
                        wq(ci, (0 if is_q else C) + ob * 128, 128),
                        xts[tch][:, ci * 512 : (ci + 1) * 512],
                        start=(ci == 0),
                        stop=(ci == CB - 1),
                    )
                if is_q:
                    nc.vector.tensor_copy(
                        qT_sb[:, ob * NQ + t0 : ob * NQ + t0 + 512], psv[:, :]
                    )
                else:
                    nc.vector.tensor_copy(
                        kT_sb[:, ob * N + t0 : ob * N + t0 + 512], psv[:, :]
                    )

            def v_unit(t128, o0, w):
                """one v unit: 128 tokens x [o0, o0+w) v-dims, written
                (bf16) into the vaug slot layout"""
                tch, tb = divmod(t128, 4)
                psv = ps2.tile(
                    [128, 512], F32, tag="psV", bufs=2, name=f"v{t128}_{o0}"
                )
                for ci in range(CB):
                    nc.tensor.matmul(
                        psv[:, :w],
                        xts[tch][:, ci * 512 + tb * 128 : ci * 512 + (tb + 1) * 128],
                        wq(ci, 2 * C + o0, w),
                        start=(ci == 0),
                        stop=(ci == CB - 1),
                    )
                nh = w // D
                src = psv[:, :w].rearrange("p (h x) -> p h x", x=D)
                h0 = o0 // D
                base = t128 * VW + h0 * (D + 1)
                dst = vaug_sb[:, base : base + nh * (D + 1)].rearrange(
                    "p (h x) -> p h x", x=D + 1
                )[:, :, :D]
                nc.vector.tensor_copy(dst, src)

            # remaining k/q blocks, drained by the pass fillers in order;
            # block hb is always complete before pair hb's first pass
            kq_queue = []
            for ob in range(1, CB):
                for tch in range(4):
                    kq_queue.append((ob, tch, False))
                for tch in range(2):
                    kq_queue.append((ob, tch, True))

            def fill_kq():
                if kq_queue:
                    ob_, tch_, is_q_ = kq_queue.pop(0)
                    kq_unit(ob_, tch_, is_q_)

            # ---- attention machinery ----------------------------------
            def epi_pe(hb_, qc_, outs_):
                """PE part of a pass's normalization epilogue. The two
                heads' 1/denom broadcasts go to different column strips of
                one PSUM tile (col tiling) so they run concurrently."""
                psb = ps2.tile(
                    [128, 512], F32, tag="psV", bufs=2,
                    name=f"psb{hb_}_{qc_}",
                )
                for hh_ in range(2):
                    nc.tensor.matmul(
                        psb[64 * hh_ : 64 * hh_ + 64, :],
                        ones_sb[:, :],
                        outs_[hh_][1][:, :],
                        start=True,
                        stop=True,
                    )
                for hh_ in range(2):
                    nc.vector.tensor_mul(
                        attnT_sb[
                            64 * hh_ : 64 * hh_ + 64,
                            hb_ * NQ + qc_ * 512 : hb_ * NQ + (qc_ + 1) * 512,
                        ],
                        psb[64 * hh_ : 64 * hh_ + 64, :],
                        outs_[hh_][0][:, :],
                    )

            def emit_pass(hb, qc, pend, filler=None):
                """One (head pair, query half) attention pass."""
                q0 = hb * NQ + qc * 512
                accs = [
                    ps2.tile(
                        [128, 512], F32, tag="psA", bufs=2,
                        name=f"acc{hb}_{qc}_{i}",
                    )
                    for i in range(2)
                ]
                def av_mms(pkb, ppb):
                    for hh in range(2):
                        vs = pkb * VW + (2 * hb + hh) * (D + 1)
                        nc.tensor.matmul(
                            accs[hh][0:65, :],
                            vaug_sb[:, vs : vs + D + 1],
                            ppb[:, hh * 512 : (hh + 1) * 512],
                            start=(pkb == 0),
                            stop=(pkb == TB - 1),
                        )

                # two k-blocks per step: the 4 score matmuls form an
                # alternating row-group run so their weight loads pipeline
                prev = []
                for kb2 in range(0, TB, 2):
                    scs = []
                    for kb in (kb2, kb2 + 1):
                        sc = ps2.tile(
                            [128, NQ], F32, tag="psS", bufs=2,
                            name=f"sc{hb}_{qc}_{kb}",
                        )
                        for hh in range(2):
                            p0 = 64 * hh
                            nc.tensor.matmul(
                                sc[:, hh * 512 : (hh + 1) * 512],
                                kT_sb[
                                    p0 : p0 + 64,
                                    hb * N + kb * 128 : hb * N + (kb + 1) * 128,
                                ],
                                qT_sb[p0 : p0 + 64, q0 : q0 + 512],
                                start=True,
                                stop=True,
                            )
                        scs.append(sc)
                    if filler is not None:
                        filler(kb2)
                        filler(kb2 + 1)
                    for pkb, ppb in prev:
                        av_mms(pkb, ppb)
                    prev = []
                    for i, kb in enumerate((kb2, kb2 + 1)):
                        pb = hp.tile([128, NQ], BF16, tag="probs")
                        nc.scalar.activation(
                            pb[:, :], scs[i][:, :], EXP, scale=SCALE
                        )
                        prev.append((kb, pb))
                    if kb2 == 2 and pend is not None:
                        epi_pe(*pend)
                        pend = None
                # drain attn@v for the last two k-blocks
                for pkb, ppb in prev:
                    av_mms(pkb, ppb)
                # epilogue DVE part: drain accumulators + 1/denominator
                outs = []
                for hh in range(2):
                    acc = accs[hh]
                    cpy = mi.tile([64, 512], F32, tag="cpy")
                    nc.vector.tensor_copy(cpy[:, :], acc[0:64, :])
                    den = mi.tile([1, 512], F32, tag="den")
                    nc.vector.tensor_copy(den[:, :], acc[64:65, :])
                    rec = mi.tile([1, 512], F32, tag="rec")
                    nc.vector.reciprocal_approx_fast(rec[:, :], den[:, :])
                    row = mi.tile([1, 512], BF16, tag="row")
                    nc.vector.tensor_copy(row[:, :], rec[:, :])
                    outs.append((cpy, row))
                return (hb, qc, outs)

            # ---- pre-phase: k/q blocks for head pair 0 ----------------
            for tch in range(4):
                kq_unit(0, tch, False)
                if tch < 2:
                    kq_unit(0, tch, True)

            # ---- phase 2: 12 passes -----------------------------------
            # pass 0 produces v just-in-time (block kb in step kb, one
            # step before attn@v needs it); passes 1-5 drain kq_queue
            def fill_v(kb):
                v_unit(kb, 0, 512)
                v_unit(kb, 512, 256)
                if kb % 8 == 7:
                    fill_kq()

            def fill_k(kb):
                if kb % 3 == 0:
                    fill_kq()

            pend = emit_pass(0, 0, None, filler=fill_v)
            for pi in range(1, 2 * HB):
                hb, qc = divmod(pi, 2)
                filler = fill_k if pi <= 5 else None
                pend = emit_pass(hb, qc, pend, filler=filler)
            epi_pe(*pend)
            assert not kq_queue

            wqxt[1].__exit__(None, None, None)
            wqxt[0].__exit__(None, None, None)

            # ---- phase 3: output projection (out^T form) --------------
            for ob in range(CB):
                psp = ps2.tile([128, NQ], F32, tag="psS", name=f"prj{ob}")
                for cb in range(CB):
                    for qc in range(2):
                        nc.tensor.matmul(
                            psp[:, qc * 512 : (qc + 1) * 512],
                            wproj_sb[:, cb * C + ob * 128 : cb * C + (ob + 1) * 128],
                            attnT_sb[:, cb * NQ + qc * 512 : cb * NQ + (qc + 1) * 512],
                            start=(cb == 0),
                            stop=(cb == CB - 1),
                        )
                ot = op_pool.tile([128, NQ], BF16, tag="out")
                nc.vector.tensor_scalar_add(
                    ot[:, :], psp[:, :], bias_sb[:, ob : ob + 1]
                )
                nc.sync.dma_start(
                    out=outT[ob * 128 : (ob + 1) * 128, :], in_=ot[:, :]
                )

    nc.finalize()
    return nc


_NC_CACHE = []


def _get_nc():
    if not _NC_CACHE:
        _NC_CACHE.append(_build())
    return _NC_CACHE[0]


def kernel(x, w_qkv, w_proj, b_proj):
    x = np.asarray(x, dtype=np.float32)
    w_qkv = np.asarray(w_qkv, dtype=np.float32)
    w_proj = np.asarray(w_proj, dtype=np.float32)
    b_proj = np.asarray(b_proj, dtype=np.float32)

    nc = _get_nc()

    wqkvT = w_qkv.T.astype(ml_dtypes.bfloat16)  # [C, 3C]
    wq3 = np.ascontiguousarray(wqkvT).reshape(CB, 128, 3 * C)  # [ci, p, o]
    wqp = np.concatenate(
        [
            wq3[:, :, o0 : o0 + w].transpose(1, 0, 2).reshape(128, CB * w)
            for o0, w in _WQ_GROUPS
        ],
        axis=1,
    )
    wqp = np.ascontiguousarray(wqp)
    wprojT = np.ascontiguousarray(w_proj.T).astype(ml_dtypes.bfloat16)
    bias = np.ascontiguousarray(b_proj.reshape(C, 1))

    in_maps = []
    for core in range(N_CORES):
        b, half = divmod(core, 2)
        # own 1024 query tokens first, then the other half (key order
        # within attention is permutation-invariant)
        mine = x[b, half * NQ : (half + 1) * NQ].T
        other = x[b, (1 - half) * NQ : (2 - half) * NQ].T
        xTc = np.concatenate([mine, other], axis=1).astype(ml_dtypes.bfloat16)
        # pack to the SBUF image: cols = [tch][ci][t]
        xTp = np.ascontiguousarray(
            xTc.reshape(CB, 128, 4, 512).transpose(1, 2, 0, 3).reshape(128, CB * N)
        )
        in_maps.append({"xTp": xTp, "wqp": wqp, "wprojT": wprojT, "bias": bias})

    res = run_bass_kernel_spmd(nc, in_maps, core_ids=list(range(N_CORES)))

    out = np.empty((B, N, C), dtype=np.float32)
    for core in range(N_CORES):
        b, half = divmod(core, 2)
        out[b, half * NQ : (half + 1) * NQ, :] = (
            res.results[core]["outT"].astype(np.float32).T
        )
    return out


# revision 29
# speedup vs baseline: 1.0124x; 1.0124x over previous
"""Distributed multi-head attention for TRN2 (8 NeuronCores).

Reference computation (per batch b):
    qkv = x @ w_qkv.T                         # (N, 3C)
    q, k, v = split/reshape to (H, N, D)
    attn = softmax(q @ k.T * D**-0.5)         # per head
    out = (attn @ v) reassembled to (N, C)
    out = out @ w_proj.T + b_proj

Sharding: 8 cores = 4 batches x 2 query-halves. Each core computes k/v
for all 2048 tokens of its batch (duplicated across the 2 cores of a
batch - cheaper than communicating), q for its own 1024 tokens, the
full attention for all 12 heads over its 1024 queries, and the output
projection. No collectives.

Layout strategy (all chosen so no on-chip transposes are needed):
  - host passes x^T and w_qkv^T so projections contract over partitions
  - q,k are produced "d-major" ([head-dim, tokens]) via out^T-form
    matmuls; scores are computed transposed ([keys, queries]) which is
    exactly the layout attn@v consumes as its stationary-side operand
  - softmax needs no max-subtraction (scores ~ N(0,1), fp32 exp range)
  - the denominator rides along as a ones-column appended to v (M=65
    matmuls); normalization uses a K=1 ones-matmul to broadcast 1/denom
    across partitions
  - all matmuls in bf16 (PSUM accumulation is fp32); softmax exp runs
    on the scalar (ACT) engine from PSUM f32, writing bf16 probs

Schedule: the ACT engine (softmax exp, ~1us per 128x1024 tile) is the
steady-state bottleneck; everything else hides under it. Attention runs
as 12 passes (head pair x query half). Per pass and k-block: the two
heads' score matmuls write one shared PSUM tile, alternating PE row
groups (base partition 0/64) so they run concurrently; exp(kb) overlaps
scores(kb+1) via two PSUM slots; attn@v lags by one k-block. The query
halving keeps the pass's PSUM footprint at 6 banks, leaving 2 banks for
"filler" projection work that keeps the PE busy (and its HAM clock
warm): pass 0 produces v block kb just-in-time in step kb, passes 1-5
drain the k/q blocks of later pairs. The per-pass normalization
epilogue is split so its PE part lands inside the next pass.

Self-contained: hardcodes B=4, N=2048, C=768, H=12, D=64.
"""

import numpy as np
import ml_dtypes

import concourse.bass as bass
import concourse.mybir as mybir
from concourse import bacc
from concourse.tile import TileContext
from concourse.bass_utils import run_bass_kernel_spmd

F32 = mybir.dt.float32
BF16 = mybir.dt.bfloat16
EXP = mybir.ActivationFunctionType.Exp

B, N, C = 4, 2048, 768
H, D = 12, 64
SCALE = float(D) ** -0.5  # 0.125
NQ = N // 2  # queries per core: 1024
CB = C // 128  # 6 c-chunks
TB = N // 128  # 16 token blocks
HB = H // 2  # 6 head pairs
VW = H * (D + 1)  # 780: v block width with ones columns

N_CORES = 8

# w_qkv columns, grouped in the order the projection units consume them:
# pair-0 k/q, all v, then k/q for pairs 1..5. Each group holds its column
# range for all six 128-row input chunks, contiguously.
_WQ_GROUPS = [(C, 128), (0, 128), (2 * C, C)]
for _ob in range(1, CB):
    _WQ_GROUPS.append((C + _ob * 128, 128))
    _WQ_GROUPS.append((_ob * 128, 128))
_WQ_BASE = {}
_cur = 0
for _o0, _w in _WQ_GROUPS:
    _WQ_BASE[_o0] = (_cur, _w)
    _cur += CB * _w


def _build():
    nc = bacc.Bacc(None, target_bir_lowering=False)

    # host-packed SBUF images: xTp cols = [tch][ci][t]; wqp cols grouped
    # in consumption order (see _WQ_GROUPS)
    xTp = nc.declare_dram_parameter("xTp", [128, CB * N], BF16, isOutput=False)
    wqp = nc.declare_dram_parameter("wqp", [128, CB * 3 * C], BF16, isOutput=False)
    wprojT = nc.declare_dram_parameter("wprojT", [C, C], BF16, isOutput=False)
    bias = nc.declare_dram_parameter("bias", [C, 1], F32, isOutput=False)
    outT = nc.declare_dram_parameter("outT", [C, NQ], BF16, isOutput=True)

    with TileContext(nc) as tc:
        with (
            tc.tile_pool(name="per", bufs=1) as per,
            tc.tile_pool(name="p23", bufs=1) as p23,
            tc.tile_pool(name="hp", bufs=8) as hp,
            tc.tile_pool(name="mi", bufs=3) as mi,
            tc.tile_pool(name="op", bufs=2) as op_pool,
            tc.tile_pool(name="ps", bufs=2, space="PSUM") as ps2,
        ):
            # ---- persistent tiles -------------------------------------
            qT_sb = per.tile([128, CB * NQ], BF16)  # q^T  [2 heads/blk, 1024]
            kT_sb = per.tile([128, CB * N], BF16)  # k^T  [2 heads/blk, 2048]
            vaug_sb = per.tile([128, TB * VW], BF16)  # v + ones cols
            bias_sb = per.tile([128, CB], F32)
            ones_sb = per.tile([1, 64], BF16)
            attnT_sb = p23.tile([128, CB * NQ], BF16)  # attn out^T
            wproj_sb = p23.tile([128, CB * C], BF16)

            nc.vector.memset(ones_sb[:, :], 1.0)
            # ones columns of vaug: col 64 of each 65-wide head slot
            vaug_ones = vaug_sb[:, :].rearrange(
                "p (t h x) -> p t h x", t=TB, h=H, x=D + 1
            )[:, :, :, D : D + 1]
            nc.vector.memset(vaug_ones, 1.0)

            # weights + activations pools, closed once the projection
            # filler has consumed them
            wqxt = (tc.tile_pool(name="wq", bufs=1), tc.tile_pool(name="xt", bufs=4))
            wq_pool = wqxt[0].__enter__()
            xt_pool = wqxt[1].__enter__()

            wqkv_sb = wq_pool.tile([128, CB * 3 * C], BF16)
            xts = [
                xt_pool.tile([128, CB * 512], BF16, tag="xt", name=f"xt{t}")
                for t in range(4)
            ]

            def _dma_xt(tch):
                nc.sync.dma_start(
                    out=xts[tch][:, :],
                    in_=xTp[:, tch * CB * 512 : (tch + 1) * CB * 512],
                )

            def _dma_wq(gi):
                o0, w = _WQ_GROUPS[gi]
                base, _ = _WQ_BASE[o0]
                nc.sync.dma_start(
                    out=wqkv_sb[:, base : base + CB * w],
                    in_=wqp[:, base : base + CB * w],
                )

            # consumption order: chunk 0 + pair-0 k/q cols first, then the
            # remaining token chunks, v cols, later pairs' k/q cols
            _dma_xt(0)
            _dma_wq(0)
            _dma_wq(1)
            for t in range(1, 4):
                _dma_xt(t)
            for gi in range(2, len(_WQ_GROUPS)):
                _dma_wq(gi)

            def wq(ci, o0, width):
                if o0 >= 2 * C:
                    base, gw = _WQ_BASE[2 * C]
                    off = o0 - 2 * C
                else:
                    base, gw = _WQ_BASE[o0]
                    off = 0
                return wqkv_sb[:, base + ci * gw + off : base + ci * gw + off + width]

            # phase-2/3-only weights: after the critical-path DMAs
            for ci in range(CB):
                nc.sync.dma_start(
                    out=bias_sb[:, ci : ci + 1],
                    in_=bias[ci * 128 : (ci + 1) * 128, :],
                )
                nc.sync.dma_start(
                    out=wproj_sb[:, ci * C : (ci + 1) * C],
                    in_=wprojT[ci * 128 : (ci + 1) * 128, :],
                )

            # ---- projection work units (PE filler) --------------------
            def kq_unit(ob, tch, is_q):
                """one k^T (or q^T) block: out-dims block ob, 512 tokens"""
                t0 = tch * 512
                kind = "q" if is_q else "k"
                psv = ps2.tile(
                    [128, 512], F32, tag="psV", bufs=2, name=f"{kind}{ob}_{tch}"
                )
                for ci in range(CB):
                    nc.tensor.matmul(
                        psv[:, :],
                        wq(ci, (0 if is_q else C) + ob * 128, 128),
                        xts[tch][:, ci * 512 : (ci + 1) * 512],
                        start=(ci == 0),
                        stop=(ci == CB - 1),
                    )
                if is_q:
                    nc.vector.tensor_copy(
                        qT_sb[:, ob * NQ + t0 : ob * NQ + t0 + 512], psv[:, :]
                    )
                else:
                    nc.vector.tensor_copy(
                        kT_sb[:, ob * N + t0 : ob * N + t0 + 512], psv[:, :]
                    )

            def v_unit(t128, o0, w):
                """one v unit: 128 tokens x [o0, o0+w) v-dims, written
                (bf16) into the vaug slot layout"""
                tch, tb = divmod(t128, 4)
                psv = ps2.tile(
                    [128, 512], F32, tag="psV", bufs=2, name=f"v{t128}_{o0}"
                )
                for ci in range(CB):
                    nc.tensor.matmul(
                        psv[:, :w],
                        xts[tch][:, ci * 512 + tb * 128 : ci * 512 + (tb + 1) * 128],
                        wq(ci, 2 * C + o0, w),
                        start=(ci == 0),
                        stop=(ci == CB - 1),
                    )
                nh = w // D
                src = psv[:, :w].rearrange("p (h x) -> p h x", x=D)
                h0 = o0 // D
                base = t128 * VW + h0 * (D + 1)
                dst = vaug_sb[:, base : base + nh * (D + 1)].rearrange(
                    "p (h x) -> p h x", x=D + 1
                )[:, :, :D]
                nc.vector.tensor_copy(dst, src)

            # remaining k/q blocks, drained by the pass fillers in order;
            # block hb is always complete before pair hb's first pass
            kq_queue = []
            for ob in range(1, CB):
                for tch in range(4):
                    kq_queue.append((ob, tch, False))
                for tch in range(2):
                    kq_queue.append((ob, tch, True))

            def fill_kq():
                if kq_queue:
                    ob_, tch_, is_q_ = kq_queue.pop(0)
                    kq_unit(ob_, tch_, is_q_)

            # ---- attention machinery ----------------------------------
            def epi_pe(hb_, qc_, outs_):
                """PE part of a pass's normalization epilogue. The two
                heads' 1/denom broadcasts go to different column strips of
                one PSUM tile (col tiling) so they run concurrently."""
                psb = ps2.tile(
                    [128, 512], F32, tag="psV", bufs=2,
                    name=f"psb{hb_}_{qc_}",
                )
                for hh_ in range(2):
                    nc.tensor.matmul(
                        psb[64 * hh_ : 64 * hh_ + 64, :],
                        ones_sb[:, :],
                        outs_[hh_][1][:, :],
                        start=True,
                        stop=True,
                    )
                for hh_ in range(2):
                    nc.vector.tensor_mul(
                        attnT_sb[
                            64 * hh_ : 64 * hh_ + 64,
                            hb_ * NQ + qc_ * 512 : hb_ * NQ + (qc_ + 1) * 512,
                        ],
                        psb[64 * hh_ : 64 * hh_ + 64, :],
                        outs_[hh_][0][:, :],
                    )

            def emit_pass(hb, qc, pend, filler=None):
                """One (head pair, query half) attention pass."""
                q0 = hb * NQ + qc * 512
                accs = [
                    ps2.tile(
                        [128, 512], F32, tag="psA", bufs=2,
                        name=f"acc{hb}_{qc}_{i}",
                    )
                    for i in range(2)
                ]
                def av_mms(pkb, ppb):
                    for hh in range(2):
                        vs = pkb * VW + (2 * hb + hh) * (D + 1)
                        nc.tensor.matmul(
                            accs[hh][0:65, :],
                            vaug_sb[:, vs : vs + D + 1],
                            ppb[:, hh * 512 : (hh + 1) * 512],
                            start=(pkb == 0),
                            stop=(pkb == TB - 1),
                        )

                # two k-blocks per step: the 4 score matmuls form an
                # alternating row-group run so their weight loads pipeline
                prev = []
                for kb2 in range(0, TB, 2):
                    scs = []
                    for kb in (kb2, kb2 + 1):
                        sc = ps2.tile(
                            [128, NQ], F32, tag="psS", bufs=2,
                            name=f"sc{hb}_{qc}_{kb}",
                        )
                        for hh in range(2):
                            p0 = 64 * hh
                            nc.tensor.matmul(
                                sc[:, hh * 512 : (hh + 1) * 512],
                                kT_sb[
                                    p0 : p0 + 64,
                                    hb * N + kb * 128 : hb * N + (kb + 1) * 128,
                                ],
                                qT_sb[p0 : p0 + 64, q0 : q0 + 512],
                                start=True,
                                stop=True,
                            )
                        scs.append(sc)
                    if filler is not None:
                        filler(kb2)
                        filler(kb2 + 1)
                    for pkb, ppb in prev:
                        av_mms(pkb, ppb)
                    prev = []
                    for i, kb in enumerate((kb2, kb2 + 1)):
                        pb = hp.tile([128, NQ], BF16, tag="probs")
                        nc.scalar.activation(
                            pb[:, :], scs[i][:, :], EXP, scale=SCALE
                        )
                        prev.append((kb, pb))
                    if kb2 == 2 and pend is not None:
                        epi_pe(*pend)
                        pend = None
                # drain attn@v for the last two k-blocks
                for pkb, ppb in prev:
                    av_mms(pkb, ppb)
                # epilogue DVE part: drain accumulators + 1/denominator
                outs = []
                for hh in range(2):
                    acc = accs[hh]
                    cpy = mi.tile([64, 512], F32, tag="cpy")
                    nc.vector.tensor_copy(cpy[:, :], acc[0:64, :])
                    den = mi.tile([1, 512], F32, tag="den")
                    nc.vector.tensor_copy(den[:, :], acc[64:65, :])
                    rec = mi.tile([1, 512], F32, tag="rec")
                    nc.vector.reciprocal_approx_fast(rec[:, :], den[:, :])
                    row = mi.tile([1, 512], BF16, tag="row")
                    nc.vector.tensor_copy(row[:, :], rec[:, :])
                    outs.append((cpy, row))
                return (hb, qc, outs)

            # ---- pre-phase: k/q blocks for head pair 0 ----------------
            for tch in range(4):
                kq_unit(0, tch, False)
                if tch < 2:
                    kq_unit(0, tch, True)

            # ---- phase 2: 12 passes -----------------------------------
            # pass 0 produces v just-in-time (block kb in step kb, one
            # step before attn@v needs it); passes 1-5 drain kq_queue
            def fill_v(kb):
                v_unit(kb, 0, 512)
                v_unit(kb, 512, 256)
                if kb % 8 == 7:
                    fill_kq()

            def fill_k(kb):
                if kb % 3 == 0:
                    fill_kq()

            pend = emit_pass(0, 0, None, filler=fill_v)
            for pi in range(1, 2 * HB):
                hb, qc = divmod(pi, 2)
                filler = fill_k if pi <= 5 else None
                pend = emit_pass(hb, qc, pend, filler=filler)
            epi_pe(*pend)
            assert not kq_queue

            wqxt[1].__exit__(None, None, None)
            wqxt[0].__exit__(None, None, None)

            # ---- phase 3: output projection (out^T form) --------------
            for ob in range(CB):
                psp = ps2.tile([128, NQ], F32, tag="psS", name=f"prj{ob}")
                for cb in range(CB):
                    for qc in range(2):
                        nc.tensor.matmul(
                            psp[:, qc * 512 : (qc + 1) * 512],
                            wproj_sb[:, cb * C + ob * 128 : cb * C + (ob + 1) * 128],
                            attnT_sb[:, cb * NQ + qc * 512 : cb * NQ + (qc + 1) * 512],
                            start=(cb == 0),
                            stop=(cb == CB - 1),
                        )
                ot = op_pool.tile([128, NQ], BF16, tag="out")
                nc.vector.tensor_scalar_add(
                    ot[:, :], psp[:, :], bias_sb[:, ob : ob + 1]
                )
                nc.sync.dma_start(
                    out=outT[ob * 128 : (ob + 1) * 128, :], in_=ot[:, :]
                )

    nc.finalize()
    return nc


_NC_CACHE = []


def _get_nc():
    if not _NC_CACHE:
        _NC_CACHE.append(_build())
    return _NC_CACHE[0]


def kernel(x, w_qkv, w_proj, b_proj):
    x = np.asarray(x, dtype=np.float32)
    w_qkv = np.asarray(w_qkv, dtype=np.float32)
    w_proj = np.asarray(w_proj, dtype=np.float32)
    b_proj = np.asarray(b_proj, dtype=np.float32)

    nc = _get_nc()

    wqkvT = w_qkv.T.astype(ml_dtypes.bfloat16)  # [C, 3C]
    wq3 = np.ascontiguousarray(wqkvT).reshape(CB, 128, 3 * C)  # [ci, p, o]
    wqp = np.concatenate(
        [
            wq3[:, :, o0 : o0 + w].transpose(1, 0, 2).reshape(128, CB * w)
            for o0, w in _WQ_GROUPS
        ],
        axis=1,
    )
    wqp = np.ascontiguousarray(wqp)
    wprojT = np.ascontiguousarray(w_proj.T).astype(ml_dtypes.bfloat16)
    bias = np.ascontiguousarray(b_proj.reshape(C, 1))

    in_maps = []
    for core in range(N_CORES):
        b, half = divmod(core, 2)
        # own 1024 query tokens first, then the other half (key order
        # within attention is permutation-invariant)
        mine = x[b, half * NQ : (half + 1) * NQ].T
        other = x[b, (1 - half) * NQ : (2 - half) * NQ].T
        xTc = np.concatenate([mine, other], axis=1).astype(ml_dtypes.bfloat16)
        # pack to the SBUF image: cols = [tch][ci][t]
        xTp = np.ascontiguousarray(
            xTc.reshape(CB, 128, 4, 512).transpose(1, 2, 0, 3).reshape(128, CB * N)
        )
        in_maps.append({"xTp": xTp, "wqp": wqp, "wprojT": wprojT, "bias": bias})

    res = run_bass_kernel_spmd(nc, in_maps, core_ids=list(range(N_CORES)))

    out = np.empty((B, N, C), dtype=np.float32)
    for core in range(N_CORES):
        b, half = divmod(core, 2)
        out[b, half * NQ : (half + 1) * NQ, :] = (
            res.results[core]["outT"].astype(np.float32).T
        )
    return out
